# revision 11
# baseline (speedup 1.0000x reference)
"""Trainium2 Bass kernel for nn_DenseAttentionOneHead.

reference:  q = X @ W^T ; pre = q @ X^T ; out = pre @ X     (X [2,4096,1024])
All linear, so out_b = (X_b W^T)(X_b^T X_b) = Q_b S_b with
  Q_b = X_b W^T,  S_b = X_b^T X_b  ([D,D], summed over rows -> AllReduce).

Sharding (8 cores): cores 0-3 handle batch 0, cores 4-7 batch 1; each core owns
1024 rows of its batch.  The host passes each core its shard Xs, the transposed
shard Xs^T and W^T (host transposes are free), so the device does pure matmuls.

S is symmetric: only the 36 upper-triangle 128x128 blocks are computed (waves
of PSUM accumulators, chunk-outer so the PE starts ~2us in), cast to fp16,
packed and AllReduced in a single collective per 4-core group.  Q^T = W Xs^T
runs while the collective flies.  S (fp32r) is then reconstructed — direct
casts for upper blocks, f16 PE transposes for mirrored ones — and
out = Q S (lhsT = Q^T blocks) finishes the kernel.
"""

import numpy as np

import concourse.mybir as mybir
import concourse.tile as tile
from concourse import bacc
from concourse.bass_utils import run_bass_kernel_spmd
from concourse.masks import make_identity

F32 = mybir.dt.float32
F32R = mybir.dt.float32r
F16 = mybir.dt.float16
P = 128
D = 1024
B = 2
N = 4096
NCORES = 8
GROUP = 4          # cores per batch
ROWS = N // GROUP  # 1024 rows per core
NO = D // P        # 8
RO = ROWS // P     # 8 row-chunks per core
H = 512            # column half width

# Upper-triangle block packing: block (et, dt), et <= dt, 36 blocks total.
_PREFIX = [0, 8, 15, 21, 26, 30, 33, 35]
NB = 36


def _bidx(et, dt):
    assert et <= dt
    return _PREFIX[et] + dt - et

# S-phase waves: (et, col_start, width) segments, 128-aligned, packed-contiguous
_S_WAVES = [
    [(0, 0, 512), (0, 512, 512), (1, 128, 512), (1, 640, 384)],      # b 0..14
    [(2, 256, 512), (2, 768, 256), (3, 384, 512), (3, 896, 128)],    # b 15..25
    [(4, 512, 512), (5, 640, 384), (6, 768, 256), (7, 896, 128)],    # b 26..35
]

_compiled = None


def _build():
    nc = bacc.Bacc(None, target_bir_lowering=False, debug=False, num_devices=NCORES)

    xs = nc.dram_tensor("xs", [ROWS, D], F32, kind="ExternalInput")
    xst = nc.dram_tensor("xst", [D, ROWS], F32, kind="ExternalInput")
    wt = nc.dram_tensor("wt", [D, D], F32, kind="ExternalInput")
    o_out = nc.dram_tensor("o_out", [ROWS, D], F32, kind="ExternalOutput")

    s_b = nc.dram_tensor("s_b", [NB, P, P], F16)
    s_r = nc.dram_tensor("s_r", [NB, P, P], F16)

    with tile.TileContext(nc) as tc:
        with (
            tc.tile_pool(name="big", bufs=1) as big,
            tc.tile_pool(name="stage", bufs=4) as stage,
            tc.tile_pool(name="shwp", bufs=2) as shwp,
            tc.tile_pool(name="psum", bufs=6, space="PSUM") as psum,
            tc.tile_pool(name="psum_t", bufs=2, space="PSUM") as psum_t,
        ):
            A = big.tile([P, RO, D], F32R, tag="A")        # Xs   [n, d]
            WT = big.tile([P, NO, D], F32R, tag="WT")      # W^T  [d, e]
            At = big.tile([P, NO, ROWS], F32R, tag="At")   # Xs^T [d, n]
            Qt = big.tile([P, NO, ROWS], F32R, tag="Qt")   # Q^T  [e, n]
            S = big.tile([P, NO, D], F32R, tag="A")        # reuses A's buffer
            sp = big.tile([P, NB * P], F16, tag="sp")      # packed reduced S

            ident_f = stage.tile([P, P], F32, tag="ident_f")
            make_identity(nc, ident_f)
            ident16 = stage.tile([P, P], F16, tag="ident16")
            nc.vector.tensor_copy(ident16[:], ident_f[:])

            # Per-chunk loads so the first matmuls start early
            for ch in range(RO):
                nc.sync.dma_start(
                    A[:, ch, :], xs[ch * P : (ch + 1) * P, :].bitcast(F32R)
                )
            for ch in range(NO):
                nc.sync.dma_start(
                    At[:, ch, :], xst[ch * P : (ch + 1) * P, :].bitcast(F32R)
                )
                nc.sync.dma_start(
                    WT[:, ch, :], wt[ch * P : (ch + 1) * P, :].bitcast(F32R)
                )

            # ---- S_part upper-triangle blocks, chunk-outer accumulation
            for wave in _S_WAVES:
                widths = [w for (_, _, w) in wave]
                tot = sum(widths)
                b0 = _bidx(wave[0][0], wave[0][1] // P)
                accs = [
                    psum.tile([P, 512], F32, tag="acc", name=f"sacc_{et}_{c0}")
                    for (et, c0, _) in wave
                ]
                for ch in range(RO):
                    for si, (et, c0, w) in enumerate(wave):
                        nc.tensor.matmul(
                            accs[si][:, :w],
                            A[:, ch, et * P : (et + 1) * P],
                            A[:, ch, c0 : c0 + w],
                            start=(ch == 0),
                            stop=(ch == RO - 1),
                        )
                shw = shwp.tile([P, 1920], F16, tag="shw")
                off = 0
                for si, (et, c0, w) in enumerate(wave):
                    nc.vector.tensor_copy(shw[:, off : off + w], accs[si][:, :w])
                    off += w
                nc.gpsimd.dma_start(
                    s_b[b0 : b0 + tot // P].rearrange("b p c -> p b c"),
                    shw[:, :tot].rearrange("p (b c) -> p b c", c=P),
                )

            # ---- single AllReduce of the packed triangle (per 4-core group)
            nc.gpsimd.collective_compute(
                "AllReduce",
                mybir.AluOpType.add,
                replica_groups=[[0, 1, 2, 3], [4, 5, 6, 7]],
                ins=[s_b[:].opt()],
                outs=[s_r[:].opt()],
            )

            # ---- Q^T[e, n] = sum_d W[e, d] Xs[n, d] (fills the AR window)
            for et in range(NO):
                for h in range(2):
                    acc = psum.tile([P, H], F32, tag="acc")
                    for ch in range(NO):
                        nc.tensor.matmul(
                            acc[:],
                            WT[:, ch, et * P : (et + 1) * P],
                            At[:, ch, h * H : (h + 1) * H],
                            start=(ch == 0),
                            stop=(ch == NO - 1),
                        )
                    nc.vector.tensor_copy(Qt[:, et, h * H : (h + 1) * H], acc[:])

            # ---- packed reduced S back to SBUF, reconstruct full S rows
            nc.gpsimd.dma_start(
                sp[:].rearrange("p (b c) -> p b c", c=P),
                s_r[:].rearrange("b p c -> p b c"),
            )
            for ch in range(NO):
                for dt in range(NO):
                    if ch <= dt:
                        b = _bidx(ch, dt)
                        nc.vector.tensor_copy(
                            S[:, ch, dt * P : (dt + 1) * P],
                            sp[:, b * P : (b + 1) * P],
                        )
                    else:
                        b = _bidx(dt, ch)
                        pt = psum_t.tile([P, P], F16, tag="pt")
                        nc.tensor.transpose(
                            pt[:], sp[:, b * P : (b + 1) * P], ident16[:]
                        )
                        nc.vector.tensor_copy(
                            S[:, ch, dt * P : (dt + 1) * P], pt[:]
                        )

            # ---- out = Q S : lhsT = Q^T blocks, rhs = S halves
            for h in range(2):
                for nt in range(RO):
                    acc = psum.tile([P, H], F32, tag="acc")
                    for ch in range(NO):
                        nc.tensor.matmul(
                            acc[:],
                            Qt[:, ch, nt * P : (nt + 1) * P],
                            S[:, ch, h * H : (h + 1) * H],
                            start=(ch == 0),
                            stop=(ch == NO - 1),
                        )
                    ot = stage.tile([P, H], F32, tag="ot")
                    nc.vector.tensor_copy(ot[:], acc[:])
                    nc.scalar.dma_start(
                        o_out[nt * P : (nt + 1) * P, h * H : (h + 1) * H], ot[:]
                    )

    nc.finalize()
    return nc


def _get_compiled():
    global _compiled
    if _compiled is None:
        _compiled = _build()
    return _compiled


def kernel(hidden_states, queries, _trace=False, _trace_cores=None):
    x = np.ascontiguousarray(np.asarray(hidden_states, dtype=np.float32))
    w = np.ascontiguousarray(np.asarray(queries, dtype=np.float32))
    assert x.shape == (B, N, D) and w.shape == (D, D)

    nc = _get_compiled()
    wt = np.ascontiguousarray(w.T)
    in_maps = []
    for c in range(NCORES):
        b, r = c // GROUP, c % GROUP
        shard = x[b, r * ROWS : (r + 1) * ROWS]
        in_maps.append(
            {"xs": shard, "xst": np.ascontiguousarray(shard.T), "wt": wt}
        )

    res = run_bass_kernel_spmd(
        nc,
        in_maps,
        core_ids=list(range(NCORES)),
        trace=_trace,
        trace_cores=_trace_cores,
    )

    out = np.empty((B, N, D), dtype=np.float32)
    for c in range(NCORES):
        b, r = c // GROUP, c % GROUP
        out[b, r * ROWS : (r + 1) * ROWS] = res.results[c]["o_out"]

    if _trace:
        kernel.last_result = res
    return out
